# revision 25
# baseline (speedup 1.0000x reference)
"""Trainium2 Bass kernel for nn_MoELayer_12403865550894.

Expert-parallel MoE, 8 experts across 8 NeuronCores, one expert per core.
The host computes the (tiny, 34 MFLOP) router in numpy as part of the
sharding step -- the sharding IS the top-k dispatch -- and hands each core
its expert's token set already compacted and transposed (d-major), plus the
expert's weights in bf16.  Each core runs a dense 3-layer MLP over exactly
max-load token columns; the host scatter-adds the compact outputs (and the
rank-1 gate*bias term) into the full [4096, 1024] result.

Layer 3 keeps w3 chunks stationary and streams gate-scaled activations
(h2g = relu(.)*gate), so the gate and the output bias commute out of the
matmul; output is produced o-major and transposed on the host.

Self-contained: depends only on the container's /opt/trn_rl_repo runtime.
"""

import sys

if "/opt/trn_rl_repo" not in sys.path:
    sys.path.insert(0, "/opt/trn_rl_repo")

import numpy as np
import ml_dtypes

import concourse.bass as bass
import concourse.mybir as mybir
import concourse.tile as tile
from concourse.bass import ts
from concourse.bass_utils import run_bass_kernel_spmd

F32 = mybir.dt.float32
BF16 = mybir.dt.bfloat16
AF = mybir.ActivationFunctionType
OP = mybir.AluOpType

N, D, H, O, E = 4096, 1024, 2048, 1024, 8
KD = D // 128          # 8 contraction chunks for layer 1
KH = H // 128          # 16 contraction chunks for layers 2/3
KO = O // 128          # 8 output chunks for layer 3
NP_BF16 = np.dtype(ml_dtypes.bfloat16)
N_WARM = 18            # PE warm-up matmuls issued during the input DMA


def _split_multi_waits(nc):
    """This container's walrus build supports one sem-wait per instruction;
    Tile emits several.  Splice single-wait nops before multi-wait insts."""
    ctr = 0
    for bb in nc.main_func.blocks:
        out = []
        for ins in bb.instructions:
            si = ins.sync_info
            if si is not None and si.on_wait and len(si.on_wait) > 1:
                waits = list(si.on_wait)
                for w in waits[:-1]:
                    ctr += 1
                    nop = mybir.InstNoOp(
                        name=f"waitsplit-{ctr}",
                        sync_info=mybir.SyncInfo(on_wait=[w], on_update=[]),
                        bass_nofuse=True,
                        engine=ins.engine,
                    )
                    nc.register_instruction(nop, overwrite=True)
                    out.append(nop)
                si.on_wait = waits[-1:]
            out.append(ins)
        bb.instructions[:] = out


def _strip_second_barrier(nc):
    """TileContext's epilogue (global drain, all-engine EVSEM-butterfly
    barrier, semaphore clear, second barrier) costs ~7us of per-engine
    event-semaphore churn.  A single execution only needs: wait for the
    global clock (all compute + output DMAs done), then clear the kernel
    semaphores so a re-execution of the loaded NEFF starts clean.  Rebuild
    the tail as exactly that, gated on the idle GpSimd engine."""
    import copy as _copy
    EP = mybir.EngineType
    for bb in nc.main_func.blocks:
        isas = [i for i in bb.instructions if isinstance(i, mybir.InstISA)]
        if not isas:
            continue
        sp_drain = next(i for i in bb.instructions
                        if isinstance(i, mybir.InstDrain) and i.engine == EP.SP)
        pool_drain = [i for i in bb.instructions
                      if isinstance(i, mybir.InstDrain) and i.engine == EP.Pool][-1]
        isa = isas[-1]
        waits = (list(sp_drain.sync_info.on_wait)
                 if sp_drain.sync_info and sp_drain.sync_info.on_wait else [])
        isa.sync_info = mybir.SyncInfo(
            on_wait=[_copy.deepcopy(w) for w in waits], on_update=[])
        bb.instructions[:] = [sp_drain, pool_drain, isa]


def build_nc(c_eff):
    """c_eff = max per-expert load (exact token columns to compute)."""
    h1w = min(512, c_eff)          # xcT DMA half widths (slice-aligned)
    h2w = c_eff - h1w
    slices = []
    t0 = 0
    while t0 < c_eff:
        slices.append((t0, min(512, c_eff - t0)))
        t0 += 512

    def xc_col(k, t):
        # xcT SBUF/DRAM column layout: k-major inside each DMA half
        if t < h1w:
            return k * h1w + t
        return KD * h1w + k * h2w + (t - h1w)

    nc = bass.Bass()

    xcT_d = nc.dram_tensor("xcT", [128, KD * c_eff], BF16, kind="ExternalInput")
    w1_d = nc.dram_tensor("w1e", [128, KH * KD * 128], BF16, kind="ExternalInput")
    w2_d = nc.dram_tensor("w2e", [128, KH * H], BF16, kind="ExternalInput")
    w3_d = nc.dram_tensor("w3e", [128, KH * O], BF16, kind="ExternalInput")
    b1_d = nc.dram_tensor("b1e", [128, KH], F32, kind="ExternalInput")
    b2_d = nc.dram_tensor("b2e", [128, KH], F32, kind="ExternalInput")
    g_d = nc.dram_tensor("gates", [1, c_eff], BF16, kind="ExternalInput")
    y_d = nc.dram_tensor("y", [KO, 128, c_eff], F32, kind="ExternalOutput")

    with tile.TileContext(nc) as tc:
        cp_cm = tc.tile_pool(name="const", bufs=1)
        cp = cp_cm.__enter__()
        ones_row = cp.tile([1, 128], BF16)
        nc.vector.memset(ones_row[:], 1.0)
        b1_sb = cp.tile([128, KH], F32)
        b2_sb = cp.tile([128, KH], F32)
        g_row = cp.tile([1, c_eff], BF16)
        gb = cp.tile([128, c_eff], BF16)   # gate broadcast across partitions
        warm = cp.tile([128, 512], BF16)
        nc.vector.memset(warm[:], 0.0)

        # ---- load order on the sync HWDGE ring sets DMA priority:
        # first xcT half (covers slice 0), first w1 quarter, rest of xcT,
        # rest of w1, then w2; w3 streams after layer 1 frees its pool.
        pA_cm = tc.tile_pool(name="pA", bufs=1, side="right")
        pA = pA_cm.__enter__()
        xcT = pA.tile([128, KD * c_eff], BF16)
        w1_sb = pA.tile([128, KH * KD * 128], BF16)
        xq = KD * h1w // 2
        wq = 2 * KD * 128
        nc.sync.dma_start(xcT[:, 0:xq], xcT_d[:, 0:xq])
        nc.sync.dma_start(w1_sb[:, 0:wq], w1_d[:, 0:wq])
        nc.sync.dma_start(xcT[:, xq : KD * h1w], xcT_d[:, xq : KD * h1w])
        nc.sync.dma_start(w1_sb[:, wq : 2 * wq], w1_d[:, wq : 2 * wq])
        nc.sync.dma_start(b1_sb[:], b1_d[:, :])
        nc.sync.dma_start(b2_sb[:], b2_d[:, :])
        nc.sync.dma_start(g_row[:], g_d[:, :])
        if h2w:
            nc.sync.dma_start(xcT[:, KD * h1w : KD * c_eff],
                              xcT_d[:, KD * h1w : KD * c_eff])
        for grp in range(1, 4):
            nc.sync.dma_start(w1_sb[:, ts(grp, 4 * KD * 128)],
                              w1_d[:, ts(grp, 4 * KD * 128)])

        pW_cm = tc.tile_pool(name="pW", bufs=1)
        pW = pW_cm.__enter__()
        w2_sb = pW.tile([128, KH * H], BF16)
        nc.sync.dma_start(w2_sb[:, 0 : KH * H // 2], w2_d[:, 0 : KH * H // 2])
        nc.sync.dma_start(w2_sb[:, KH * H // 2 :], w2_d[:, KH * H // 2 :])
        h1T = pW.tile([128, KH * c_eff], BF16)

        # ---- PE warm-up: release the HAM clock gate while inputs stream in,
        # and materialize the gate-broadcast tile on the way.
        with tc.tile_pool(name="psW", bufs=2, space="PSUM") as psW:
            psw = psW.tile([128, 512], F32, tag="warm")
            for _ in range(N_WARM):
                nc.tensor.matmul(psw[:], lhsT=warm[:, 0:128], rhs=warm[:],
                                 start=True, stop=True)

        # ---------------- layer 1: h1T[ht, t] = relu(w1.T @ xcT + b1) --------
        with tc.tile_pool(name="psL1", bufs=4, space="PSUM") as psL1:
            for ht in range(KH):
                for (t0, tw) in slices:
                    ps = psL1.tile([128, 512], F32, tag="psL1")
                    for k in range(KD):
                        nc.tensor.matmul(
                            ps[:, :tw],
                            lhsT=w1_sb[:, ht * KD * 128 + k * 128 :
                                       ht * KD * 128 + (k + 1) * 128],
                            rhs=xcT[:, xc_col(k, t0) : xc_col(k, t0) + tw],
                            start=(k == 0), stop=(k == KD - 1),
                        )
                    nc.scalar.activation(
                        h1T[:, ht * c_eff + t0 : ht * c_eff + t0 + tw],
                        ps[:, :tw], AF.Relu, bias=b1_sb[:, ht : ht + 1],
                    )

        # gate broadcast, needed only by layer 2's DVE scaling
        with tc.tile_pool(name="psG", bufs=2, space="PSUM") as psG:
            for (t0, tw) in slices:
                psg = psG.tile([128, 512], F32, tag="gb")
                nc.tensor.matmul(psg[:, :tw], lhsT=ones_row[:],
                                 rhs=g_row[:, t0 : t0 + tw],
                                 start=True, stop=True)
                nc.vector.tensor_copy(gb[:, t0 : t0 + tw], psg[:, :tw])

        pA_cm.__exit__(None, None, None)

        # w3 + gated h2 reuse the space freed by xcT/w1
        pL3_cm = tc.tile_pool(name="pL3", bufs=1, side="right")
        pL3 = pL3_cm.__enter__()
        w3_sb = pL3.tile([128, KH * O], BF16)
        nc.sync.dma_start(w3_sb[:], w3_d[:, :])
        h2gT = pL3.tile([128, KH * c_eff], BF16)

        # ------- layer 2: h2gT[gt, t] = relu(w2.T @ h1T + b2) * gate[t] ------
        with (
            tc.tile_pool(name="psL2", bufs=4, space="PSUM") as psL2,
            tc.tile_pool(name="h2tmp", bufs=3) as h2tmp,
        ):
            for gt in range(KH):
                for (t0, tw) in slices:
                    ps = psL2.tile([128, 512], F32, tag="psL2")
                    for k in range(KH):
                        nc.tensor.matmul(
                            ps[:, :tw],
                            lhsT=w2_sb[:, k * H + gt * 128 : k * H + (gt + 1) * 128],
                            rhs=h1T[:, k * c_eff + t0 : k * c_eff + t0 + tw],
                            start=(k == 0), stop=(k == KH - 1),
                        )
                    tmp = h2tmp.tile([128, 512], BF16, tag="h2tmp")
                    nc.scalar.activation(
                        tmp[:, :tw], ps[:, :tw], AF.Relu,
                        bias=b2_sb[:, gt : gt + 1],
                    )
                    nc.vector.tensor_tensor(
                        h2gT[:, gt * c_eff + t0 : gt * c_eff + t0 + tw],
                        tmp[:, :tw], gb[:, t0 : t0 + tw], op=OP.mult,
                    )

        pW_cm.__exit__(None, None, None)

        # -------- layer 3: yT[o, t] = w3.T @ h2gT  (o-major, bias on host) ---
        with (
            tc.tile_pool(name="psY", bufs=4, space="PSUM") as psY,
            tc.tile_pool(name="yp", bufs=3) as yp,
        ):
            for c8 in range(KO):
                for (t0, tw) in slices:
                    ps = psY.tile([128, 512], F32, tag="psY")
                    for k in range(KH):
                        nc.tensor.matmul(
                            ps[:, :tw],
                            lhsT=w3_sb[:, k * O + c8 * 128 : k * O + (c8 + 1) * 128],
                            rhs=h2gT[:, k * c_eff + t0 : k * c_eff + t0 + tw],
                            start=(k == 0), stop=(k == KH - 1),
                        )
                    yt = yp.tile([128, 512], F32, tag="y")
                    nc.scalar.activation(yt[:, :tw], ps[:, :tw], AF.Copy)
                    nc.scalar.dma_start(y_d[c8, :, t0 : t0 + tw], yt[:, :tw])

        pL3_cm.__exit__(None, None, None)
        cp_cm.__exit__(None, None, None)

    _strip_second_barrier(nc)
    _split_multi_waits(nc)
    return nc


_NC_CACHE = {}


def _get_nc(c_eff):
    if c_eff not in _NC_CACHE:
        _NC_CACHE[c_eff] = build_nc(c_eff)
    return _NC_CACHE[c_eff]


def _route(x, router_w, router_b):
    """Replicates reference routing on host (f64: margins are >=1e-4, far
    above both f32 and f64 matmul noise, so selection matches jax f32)."""
    logits = x.astype(np.float64) @ router_w.astype(np.float64) + router_b
    m = logits.max(1, keepdims=True)
    p = np.exp(logits - m)
    p /= p.sum(1, keepdims=True)
    top2 = np.argsort(-p, axis=1, kind="stable")[:, :2]
    tp = np.take_along_axis(p, top2, axis=1)
    gates = tp / (tp.sum(1, keepdims=True) + 1e-6)
    return top2, gates.astype(np.float32)


def _flat_chunks(w, kparts):
    """[kparts*128, M] -> [128, kparts*M] with chunk-major columns, bf16."""
    m = w.shape[1]
    return np.ascontiguousarray(
        w.astype(NP_BF16).reshape(kparts, 128, m).transpose(1, 0, 2)
        .reshape(128, kparts * m))


def make_in_maps(x, router_w, router_b, w1, b1, w2, b2, w3, b3):
    x = np.asarray(x, np.float32)
    top2, gates = _route(x, np.asarray(router_w, np.float32),
                         np.asarray(router_b, np.float32))
    xT_bf = np.ascontiguousarray(x.T.astype(NP_BF16))  # [D, N]

    tok_lists = []
    gate_lists = []
    for e in range(E):
        sel = top2 == e
        tok = np.nonzero(sel.any(1))[0]
        tok_lists.append(tok)
        gate_lists.append((gates * sel)[tok].sum(1))
    c_eff = (max(len(t) for t in tok_lists) + 3) & ~3
    h1w = min(512, c_eff)
    h2w = c_eff - h1w

    in_maps = []
    for e in range(E):
        tok, ge = tok_lists[e], gate_lists[e]
        n_e = len(tok)

        xc = np.zeros((D, c_eff), NP_BF16)
        xc[:, :n_e] = xT_bf[:, tok]
        xc3 = xc.reshape(KD, 128, c_eff)
        halves = [xc3[:, :, :h1w].transpose(1, 0, 2).reshape(128, KD * h1w)]
        if h2w:
            halves.append(
                xc3[:, :, h1w:].transpose(1, 0, 2).reshape(128, KD * h2w))
        xcT_np = np.ascontiguousarray(np.concatenate(halves, axis=1))

        g_full = np.zeros((1, c_eff), np.float32)
        g_full[0, :n_e] = ge

        w1e = np.asarray(w1[e], np.float32).astype(NP_BF16)
        w1p = w1e.reshape(KD, 128, KH, 128).transpose(1, 2, 0, 3)

        in_maps.append({
            "xcT": xcT_np,
            "w1e": np.ascontiguousarray(w1p.reshape(128, KH * KD * 128)),
            "w2e": _flat_chunks(np.asarray(w2[e], np.float32), KH),
            "w3e": _flat_chunks(np.asarray(w3[e], np.float32), KH),
            "b1e": np.ascontiguousarray(
                np.asarray(b1[e], np.float32).reshape(KH, 128).T),
            "b2e": np.ascontiguousarray(
                np.asarray(b2[e], np.float32).reshape(KH, 128).T),
            "gates": g_full.astype(NP_BF16),
        })
    return in_maps, tok_lists, gate_lists, c_eff


def kernel(x, router_w, router_b, w1, b1, w2, b2, w3, b3, _trace=False):
    in_maps, tok_lists, gate_lists, c_eff = make_in_maps(
        x, router_w, router_b, w1, b1, w2, b2, w3, b3)
    nc = _get_nc(c_eff)
    res = run_bass_kernel_spmd(nc, in_maps, list(range(E)), trace=_trace)
    out = np.zeros((N, O), np.float32)
    b3f = np.asarray(b3, np.float32)
    for e, r in enumerate(res.results):
        tok, ge = tok_lists[e], gate_lists[e]
        n_e = len(tok)
        yT = r["y"].reshape(O, c_eff)
        # per-expert token ids are unique -> fancy-index add is safe
        out[tok] += yT[:, :n_e].T + ge[:, None] * b3f[e][None, :]
    kernel.last_results = res
    return out


# revision 26
# speedup vs baseline: 1.0218x; 1.0218x over previous
"""Trainium2 Bass kernel for nn_MoELayer_12403865550894.

Expert-parallel MoE, 8 experts across 8 NeuronCores, one expert per core.
The host computes the (tiny, 34 MFLOP) router in numpy as part of the
sharding step -- the sharding IS the top-k dispatch -- and hands each core
its expert's token set already compacted and transposed (d-major), plus the
expert's weights in bf16.  Each core runs a dense 3-layer MLP over exactly
max-load token columns; the host scatter-adds the compact outputs (and the
rank-1 gate*bias term) into the full [4096, 1024] result.

Layer 3 keeps w3 chunks stationary and streams gate-scaled activations
(h2g = relu(.)*gate), so the gate and the output bias commute out of the
matmul; output is produced o-major and transposed on the host.

Self-contained: depends only on the container's /opt/trn_rl_repo runtime.
"""

import sys

if "/opt/trn_rl_repo" not in sys.path:
    sys.path.insert(0, "/opt/trn_rl_repo")

import numpy as np
import ml_dtypes

import concourse.bass as bass
import concourse.mybir as mybir
import concourse.tile as tile
from concourse.bass import ts
from concourse.bass_utils import run_bass_kernel_spmd

F32 = mybir.dt.float32
BF16 = mybir.dt.bfloat16
AF = mybir.ActivationFunctionType
OP = mybir.AluOpType

N, D, H, O, E = 4096, 1024, 2048, 1024, 8
KD = D // 128          # 8 contraction chunks for layer 1
KH = H // 128          # 16 contraction chunks for layers 2/3
KO = O // 128          # 8 output chunks for layer 3
NP_BF16 = np.dtype(ml_dtypes.bfloat16)
N_WARM = 18            # PE warm-up matmuls issued during the input DMA


def _split_multi_waits(nc):
    """This container's walrus build supports one sem-wait per instruction;
    Tile emits several.  Splice single-wait nops before multi-wait insts."""
    ctr = 0
    for bb in nc.main_func.blocks:
        out = []
        for ins in bb.instructions:
            si = ins.sync_info
            if si is not None and si.on_wait and len(si.on_wait) > 1:
                waits = list(si.on_wait)
                for w in waits[:-1]:
                    ctr += 1
                    nop = mybir.InstNoOp(
                        name=f"waitsplit-{ctr}",
                        sync_info=mybir.SyncInfo(on_wait=[w], on_update=[]),
                        bass_nofuse=True,
                        engine=ins.engine,
                    )
                    nc.register_instruction(nop, overwrite=True)
                    out.append(nop)
                si.on_wait = waits[-1:]
            out.append(ins)
        bb.instructions[:] = out


def _strip_second_barrier(nc):
    """TileContext's epilogue (global drain, all-engine EVSEM-butterfly
    barrier, semaphore clear, second barrier) costs ~7us of per-engine
    event-semaphore churn.  A single execution only needs: wait for the
    global clock (all compute + output DMAs done), then clear the kernel
    semaphores so a re-execution of the loaded NEFF starts clean.  Rebuild
    the tail as exactly that, gated on the idle GpSimd engine."""
    import copy as _copy
    EP = mybir.EngineType
    for bb in nc.main_func.blocks:
        isas = [i for i in bb.instructions if isinstance(i, mybir.InstISA)]
        if not isas:
            continue
        sp_drain = next(i for i in bb.instructions
                        if isinstance(i, mybir.InstDrain) and i.engine == EP.SP)
        pool_drain = [i for i in bb.instructions
                      if isinstance(i, mybir.InstDrain) and i.engine == EP.Pool][-1]
        isa = isas[-1]
        waits = (list(sp_drain.sync_info.on_wait)
                 if sp_drain.sync_info and sp_drain.sync_info.on_wait else [])
        isa.sync_info = mybir.SyncInfo(
            on_wait=[_copy.deepcopy(w) for w in waits], on_update=[])
        bb.instructions[:] = [sp_drain, pool_drain, isa]


def build_nc(c_eff):
    """c_eff = max per-expert load (exact token columns to compute)."""
    h1w = min(512, c_eff)          # xcT DMA half widths (slice-aligned)
    h2w = c_eff - h1w
    slices = []
    t0 = 0
    while t0 < c_eff:
        slices.append((t0, min(512, c_eff - t0)))
        t0 += 512

    def xc_col(k, t):
        # xcT SBUF/DRAM column layout: k-major inside each DMA half
        if t < h1w:
            return k * h1w + t
        return KD * h1w + k * h2w + (t - h1w)

    nc = bass.Bass()

    xcT_d = nc.dram_tensor("xcT", [128, KD * c_eff], BF16, kind="ExternalInput")
    w1_d = nc.dram_tensor("w1e", [128, KH * KD * 128], BF16, kind="ExternalInput")
    w2_d = nc.dram_tensor("w2e", [128, KH * H], BF16, kind="ExternalInput")
    w3_d = nc.dram_tensor("w3e", [128, KH * O], BF16, kind="ExternalInput")
    b1_d = nc.dram_tensor("b1e", [128, KH], F32, kind="ExternalInput")
    b2_d = nc.dram_tensor("b2e", [128, KH], F32, kind="ExternalInput")
    g_d = nc.dram_tensor("gates", [1, c_eff], BF16, kind="ExternalInput")
    y_d = nc.dram_tensor("y", [KO, 128, c_eff], F32, kind="ExternalOutput")

    with tile.TileContext(nc) as tc:
        cp_cm = tc.tile_pool(name="const", bufs=1)
        cp = cp_cm.__enter__()
        ones_row = cp.tile([1, 128], BF16)
        nc.vector.memset(ones_row[:], 1.0)
        b1_sb = cp.tile([128, KH], F32)
        nc.scalar.dma_start(b1_sb[:], b1_d[:, :])
        b2_sb = cp.tile([128, KH], F32)
        nc.scalar.dma_start(b2_sb[:], b2_d[:, :])
        g_row = cp.tile([1, c_eff], BF16)
        nc.scalar.dma_start(g_row[:], g_d[:, :])
        gb = cp.tile([128, c_eff], BF16)   # gate broadcast across partitions
        warm = cp.tile([128, 512], BF16)
        nc.vector.memset(warm[:], 0.0)

        # ---- load order on the sync HWDGE ring sets DMA priority:
        # first xcT half (covers slice 0), first w1 quarter, rest of xcT,
        # rest of w1, then w2; w3 streams after layer 1 frees its pool.
        pA_cm = tc.tile_pool(name="pA", bufs=1, side="right")
        pA = pA_cm.__enter__()
        xcT = pA.tile([128, KD * c_eff], BF16)
        w1_sb = pA.tile([128, KH * KD * 128], BF16)
        xq = KD * h1w // 2
        wq = 2 * KD * 128
        nc.sync.dma_start(xcT[:, 0:xq], xcT_d[:, 0:xq])
        nc.sync.dma_start(w1_sb[:, 0:wq], w1_d[:, 0:wq])
        nc.sync.dma_start(xcT[:, xq : KD * h1w], xcT_d[:, xq : KD * h1w])
        nc.sync.dma_start(w1_sb[:, wq : 2 * wq], w1_d[:, wq : 2 * wq])
        if h2w:
            nc.sync.dma_start(xcT[:, KD * h1w : KD * c_eff],
                              xcT_d[:, KD * h1w : KD * c_eff])
        for grp in range(1, 4):
            nc.sync.dma_start(w1_sb[:, ts(grp, 4 * KD * 128)],
                              w1_d[:, ts(grp, 4 * KD * 128)])

        pW_cm = tc.tile_pool(name="pW", bufs=1)
        pW = pW_cm.__enter__()
        w2_sb = pW.tile([128, KH * H], BF16)
        nc.sync.dma_start(w2_sb[:, 0 : KH * H // 2], w2_d[:, 0 : KH * H // 2])
        nc.sync.dma_start(w2_sb[:, KH * H // 2 :], w2_d[:, KH * H // 2 :])
        h1T = pW.tile([128, KH * c_eff], BF16)

        # ---- PE warm-up: release the HAM clock gate while inputs stream in,
        # and materialize the gate-broadcast tile on the way.
        with tc.tile_pool(name="psW", bufs=2, space="PSUM") as psW:
            psw = psW.tile([128, 512], F32, tag="warm")
            for _ in range(N_WARM):
                nc.tensor.matmul(psw[:], lhsT=warm[:, 0:128], rhs=warm[:],
                                 start=True, stop=True)
            for (t0, tw) in slices:
                psg = psW.tile([128, 512], F32, tag="gb")
                nc.tensor.matmul(psg[:, :tw], lhsT=ones_row[:],
                                 rhs=g_row[:, t0 : t0 + tw],
                                 start=True, stop=True)
                nc.vector.tensor_copy(gb[:, t0 : t0 + tw], psg[:, :tw])

        # ---------------- layer 1: h1T[ht, t] = relu(w1.T @ xcT + b1) --------
        with tc.tile_pool(name="psL1", bufs=4, space="PSUM") as psL1:
            for ht in range(KH):
                for (t0, tw) in slices:
                    ps = psL1.tile([128, 512], F32, tag="psL1")
                    for k in range(KD):
                        nc.tensor.matmul(
                            ps[:, :tw],
                            lhsT=w1_sb[:, ht * KD * 128 + k * 128 :
                                       ht * KD * 128 + (k + 1) * 128],
                            rhs=xcT[:, xc_col(k, t0) : xc_col(k, t0) + tw],
                            start=(k == 0), stop=(k == KD - 1),
                        )
                    nc.scalar.activation(
                        h1T[:, ht * c_eff + t0 : ht * c_eff + t0 + tw],
                        ps[:, :tw], AF.Relu, bias=b1_sb[:, ht : ht + 1],
                    )

        pA_cm.__exit__(None, None, None)

        # w3 + gated h2 reuse the space freed by xcT/w1
        pL3_cm = tc.tile_pool(name="pL3", bufs=1, side="right")
        pL3 = pL3_cm.__enter__()
        w3_sb = pL3.tile([128, KH * O], BF16)
        nc.sync.dma_start(w3_sb[:], w3_d[:, :])
        h2gT = pL3.tile([128, KH * c_eff], BF16)

        # ------- layer 2: h2gT[gt, t] = relu(w2.T @ h1T + b2) * gate[t] ------
        with (
            tc.tile_pool(name="psL2", bufs=4, space="PSUM") as psL2,
            tc.tile_pool(name="h2tmp", bufs=3) as h2tmp,
        ):
            for gt in range(KH):
                for (t0, tw) in slices:
                    ps = psL2.tile([128, 512], F32, tag="psL2")
                    for k in range(KH):
                        nc.tensor.matmul(
                            ps[:, :tw],
                            lhsT=w2_sb[:, k * H + gt * 128 : k * H + (gt + 1) * 128],
                            rhs=h1T[:, k * c_eff + t0 : k * c_eff + t0 + tw],
                            start=(k == 0), stop=(k == KH - 1),
                        )
                    tmp = h2tmp.tile([128, 512], BF16, tag="h2tmp")
                    nc.scalar.activation(
                        tmp[:, :tw], ps[:, :tw], AF.Relu,
                        bias=b2_sb[:, gt : gt + 1],
                    )
                    nc.vector.tensor_tensor(
                        h2gT[:, gt * c_eff + t0 : gt * c_eff + t0 + tw],
                        tmp[:, :tw], gb[:, t0 : t0 + tw], op=OP.mult,
                    )

        pW_cm.__exit__(None, None, None)

        # -------- layer 3: yT[o, t] = w3.T @ h2gT  (o-major, bias on host) ---
        with (
            tc.tile_pool(name="psY", bufs=4, space="PSUM") as psY,
            tc.tile_pool(name="yp", bufs=3) as yp,
        ):
            for c8 in range(KO):
                for (t0, tw) in slices:
                    ps = psY.tile([128, 512], F32, tag="psY")
                    for k in range(KH):
                        nc.tensor.matmul(
                            ps[:, :tw],
                            lhsT=w3_sb[:, k * O + c8 * 128 : k * O + (c8 + 1) * 128],
                            rhs=h2gT[:, k * c_eff + t0 : k * c_eff + t0 + tw],
                            start=(k == 0), stop=(k == KH - 1),
                        )
                    yt = yp.tile([128, 512], F32, tag="y")
                    nc.scalar.activation(yt[:, :tw], ps[:, :tw], AF.Copy)
                    nc.scalar.dma_start(y_d[c8, :, t0 : t0 + tw], yt[:, :tw])

        pL3_cm.__exit__(None, None, None)
        cp_cm.__exit__(None, None, None)

    _strip_second_barrier(nc)
    _split_multi_waits(nc)
    return nc


_NC_CACHE = {}


def _get_nc(c_eff):
    if c_eff not in _NC_CACHE:
        _NC_CACHE[c_eff] = build_nc(c_eff)
    return _NC_CACHE[c_eff]


def _route(x, router_w, router_b):
    """Replicates reference routing on host (f64: margins are >=1e-4, far
    above both f32 and f64 matmul noise, so selection matches jax f32)."""
    logits = x.astype(np.float64) @ router_w.astype(np.float64) + router_b
    m = logits.max(1, keepdims=True)
    p = np.exp(logits - m)
    p /= p.sum(1, keepdims=True)
    top2 = np.argsort(-p, axis=1, kind="stable")[:, :2]
    tp = np.take_along_axis(p, top2, axis=1)
    gates = tp / (tp.sum(1, keepdims=True) + 1e-6)
    return top2, gates.astype(np.float32)


def _flat_chunks(w, kparts):
    """[kparts*128, M] -> [128, kparts*M] with chunk-major columns, bf16."""
    m = w.shape[1]
    return np.ascontiguousarray(
        w.astype(NP_BF16).reshape(kparts, 128, m).transpose(1, 0, 2)
        .reshape(128, kparts * m))


def make_in_maps(x, router_w, router_b, w1, b1, w2, b2, w3, b3):
    x = np.asarray(x, np.float32)
    top2, gates = _route(x, np.asarray(router_w, np.float32),
                         np.asarray(router_b, np.float32))
    xT_bf = np.ascontiguousarray(x.T.astype(NP_BF16))  # [D, N]

    tok_lists = []
    gate_lists = []
    for e in range(E):
        sel = top2 == e
        tok = np.nonzero(sel.any(1))[0]
        tok_lists.append(tok)
        gate_lists.append((gates * sel)[tok].sum(1))
    c_eff = (max(len(t) for t in tok_lists) + 3) & ~3
    h1w = min(512, c_eff)
    h2w = c_eff - h1w

    in_maps = []
    for e in range(E):
        tok, ge = tok_lists[e], gate_lists[e]
        n_e = len(tok)

        xc = np.zeros((D, c_eff), NP_BF16)
        xc[:, :n_e] = xT_bf[:, tok]
        xc3 = xc.reshape(KD, 128, c_eff)
        halves = [xc3[:, :, :h1w].transpose(1, 0, 2).reshape(128, KD * h1w)]
        if h2w:
            halves.append(
                xc3[:, :, h1w:].transpose(1, 0, 2).reshape(128, KD * h2w))
        xcT_np = np.ascontiguousarray(np.concatenate(halves, axis=1))

        g_full = np.zeros((1, c_eff), np.float32)
        g_full[0, :n_e] = ge

        w1e = np.asarray(w1[e], np.float32).astype(NP_BF16)
        w1p = w1e.reshape(KD, 128, KH, 128).transpose(1, 2, 0, 3)

        in_maps.append({
            "xcT": xcT_np,
            "w1e": np.ascontiguousarray(w1p.reshape(128, KH * KD * 128)),
            "w2e": _flat_chunks(np.asarray(w2[e], np.float32), KH),
            "w3e": _flat_chunks(np.asarray(w3[e], np.float32), KH),
            "b1e": np.ascontiguousarray(
                np.asarray(b1[e], np.float32).reshape(KH, 128).T),
            "b2e": np.ascontiguousarray(
                np.asarray(b2[e], np.float32).reshape(KH, 128).T),
            "gates": g_full.astype(NP_BF16),
        })
    return in_maps, tok_lists, gate_lists, c_eff


def kernel(x, router_w, router_b, w1, b1, w2, b2, w3, b3, _trace=False):
    in_maps, tok_lists, gate_lists, c_eff = make_in_maps(
        x, router_w, router_b, w1, b1, w2, b2, w3, b3)
    nc = _get_nc(c_eff)
    res = run_bass_kernel_spmd(nc, in_maps, list(range(E)), trace=_trace)
    out = np.zeros((N, O), np.float32)
    b3f = np.asarray(b3, np.float32)
    for e, r in enumerate(res.results):
        tok, ge = tok_lists[e], gate_lists[e]
        n_e = len(tok)
        yT = r["y"].reshape(O, c_eff)
        # per-expert token ids are unique -> fancy-index add is safe
        out[tok] += yT[:, :n_e].T + ge[:, None] * b3f[e][None, :]
    kernel.last_results = res
    return out
